# revision 11
# baseline (speedup 1.0000x reference)
"""Trainium2 Bass kernel for nn_CNF_ODE_46093589021148.

Computes dy = mlp(cat[y,t]) and -div = -sum(v * (J^T v), -1) for a 4-layer
SiLU MLP (3->128->128->64->2), N=524288 rows, data-parallel over 8 cores.

Math: since div = v^T (J^T v) = v^T (J v), forward-mode JVP with tangent
u0 = [v, 0] replaces the VJP: per layer
    z = h_prev @ W + b ; h = silu(z) ; u = (u_prev @ W) * silu'(z)
then dy = h3 @ W4 + b4, Jv = u3 @ W4, -div = -(v0*Jv0 + v1*Jv1).

silu'(z) = 0.5*(1+th) + h*0.5*(1-th) with th = tanh(z/2), h = silu(z):
both served by the ACT engine's silu_and_others table set (one table load).

Layout: activations are feature-major [feat, batch] tiles of B=512 columns.
The scalar t is folded into layer-1 bias: b1c = b1 + t*W1[2,:], so L1 runs
with K=2 (y rows only); tangent L1 likewise (tangent of the t-column is 0).
L3 primal+tangent share one PSUM bank (partitions 0-63 / 64-127); L4
primal/tangent outputs share one bank at partitions 0-1 / 64-65.
"""

import numpy as np

import concourse.bacc as bacc
import concourse.mybir as mybir
from concourse import tile
from concourse.alu_op_type import AluOpType
from concourse.bass_utils import run_bass_kernel_spmd

F32 = mybir.dt.float32
F32R = mybir.dt.float32r
AF = mybir.ActivationFunctionType

N_CORES = 8
N_TOTAL = 524288
R = N_TOTAL // N_CORES  # rows per core
B = 512                 # batch tile (PSUM bank = 512 fp32)
G = 2                   # tiles per DMA chunk
H1, H2, H3, D = 128, 128, 64, 2

_CACHE = {}


def _r(ap):
    return ap.bitcast(F32R)


def build_module(rows=R):
    n_tiles = rows // B
    n_chunks = max(1, n_tiles // G)
    g = n_tiles // n_chunks

    nc = bacc.Bacc("TRN2", target_bir_lowering=False)

    y = nc.dram_tensor("y", [rows, 2], F32, kind="ExternalInput")
    v = nc.dram_tensor("v", [rows, 2], F32, kind="ExternalInput")
    W1 = nc.dram_tensor("W1", [3, H1], F32, kind="ExternalInput")
    W2 = nc.dram_tensor("W2", [H1, H2], F32, kind="ExternalInput")
    W3 = nc.dram_tensor("W3", [H2, H3], F32, kind="ExternalInput")
    W4 = nc.dram_tensor("W4", [H3, D], F32, kind="ExternalInput")
    b1c_d = nc.dram_tensor("b1c", [H1], F32, kind="ExternalInput")
    b1h_d = nc.dram_tensor("b1h", [H1], F32, kind="ExternalInput")
    b2c_d = nc.dram_tensor("b2c", [H2], F32, kind="ExternalInput")
    b2h_d = nc.dram_tensor("b2h", [H2], F32, kind="ExternalInput")
    b3c_d = nc.dram_tensor("b3c", [H3], F32, kind="ExternalInput")
    b3h_d = nc.dram_tensor("b3h", [H3], F32, kind="ExternalInput")
    b4_d = nc.dram_tensor("b4", [D], F32, kind="ExternalInput")
    dy = nc.dram_tensor("dy", [rows, 2], F32, kind="ExternalOutput")
    nd = nc.dram_tensor("nd", [rows, 1], F32, kind="ExternalOutput")

    with tile.TileContext(nc) as tc:
        with (
            tc.tile_pool(name="wp", bufs=1) as wp,
            tc.tile_pool(name="iop", bufs=3) as iop,
            tc.tile_pool(name="outp", bufs=2) as outp,
            tc.tile_pool(name="hp", bufs=2) as hp,
            tc.tile_pool(name="accp", bufs=2) as accp,
            tc.tile_pool(name="psz", bufs=2, space="PSUM") as psz,
            tc.tile_pool(name="psu", bufs=2, space="PSUM") as psu,
            tc.tile_pool(name="ps3", bufs=1, space="PSUM") as ps3,
            tc.tile_pool(name="psu3", bufs=1, space="PSUM") as psu3,
            tc.tile_pool(name="pst", bufs=1, space="PSUM") as pst,
            tc.tile_pool(name="psd", bufs=1, space="PSUM") as psd,
        ):
            # ---- one-time setup -------------------------------------------
            def wload(dt, shape):
                st = wp.tile(shape, F32, tag=dt.name + "_st")
                nc.sync.dma_start(st[:], dt.ap())
                wr = wp.tile(shape, F32R, tag=dt.name + "_r")
                nc.vector.tensor_copy(wr[:], st[:])
                return wr

            w1 = wload(W1, [3, H1])
            w2 = wload(W2, [H1, H2])
            w3 = wload(W3, [H2, H3])
            w4 = wload(W4, [H3, D])

            def bias_tile(dt, P):
                tl = wp.tile([P, 1], F32, tag=dt.name + "_sb")
                nc.sync.dma_start(tl[:], dt.ap().unsqueeze(-1))
                return tl

            b1c = bias_tile(b1c_d, H1)
            b1h = bias_tile(b1h_d, H1)
            b2sb = bias_tile(b2c_d, H2)
            b2h = bias_tile(b2h_d, H2)
            b3sb = bias_tile(b3c_d, H3)
            b3h = bias_tile(b3h_d, H3)
            b4sb = bias_tile(b4_d, D)

            negones_f = wp.tile([D, 1], F32)
            nc.gpsimd.memset(negones_f[:], -1.0)
            negones = wp.tile([D, 1], F32R)
            nc.vector.tensor_copy(negones[:], negones_f[:])

            yv = y.ap()
            vv = v.ap()
            dyv = dy.ap()
            ndv = nd.ap()

            def layer(z_ps, u_ps, bc, bh, out_h, out_t, P):
                """z,u: PSUM [P,B] views -> h=silu(z+b) into out_h (SBUF),
                tangent u*silu'(z+b) into out_t (SBUF)."""
                th = hp.tile([P, B], F32, tag=f"th{P}")
                nc.scalar.activation(out_h, z_ps, AF.Silu, bias=bc[:], scale=1.0)
                nc.scalar.activation(th[:], z_ps, AF.Tanh, bias=bh[:], scale=0.5)
                m = hp.tile([P, B], F32, tag=f"m{P}")
                acc = accp.tile([P, 1], F32, tag=f"acc{P}")
                nc.vector.affine_mul_reduce(
                    m[:], acc[:], th[:], out_h, scale=-0.5, bias=0.5
                )
                d = hp.tile([P, B], F32, tag=f"d{P}")
                nc.vector.affine_then_add(d[:], th[:], m[:], scale=0.5, bias=0.5)
                nc.vector.tensor_tensor(out_t, u_ps, d[:], AluOpType.mult)

            # ---- main loop ------------------------------------------------
            for c in range(n_chunks):
                CB = g * B
                r0 = c * CB
                yTf = iop.tile([2, CB], F32, tag="yTf")
                nc.sync.dma_start(yTf[:], yv[r0 : r0 + CB, :].rearrange("n d -> d n"))
                yT = iop.tile([2, CB], F32R, tag="yT")
                nc.vector.tensor_copy(yT[:], yTf[:])
                vTf = iop.tile([2, CB], F32, tag="vTf")
                nc.sync.dma_start(vTf[:], vv[r0 : r0 + CB, :].rearrange("n d -> d n"))
                vT = iop.tile([2, CB], F32R, tag="vT")
                nc.vector.tensor_copy(vT[:], vTf[:])
                dyT = outp.tile([2, CB], F32, tag="dyT")
                ndT = outp.tile([1, CB], F32, tag="ndT")

                for j in range(g):
                    sl = slice(j * B, (j + 1) * B)
                    yTs = yT[:, sl]
                    vTs = vT[:, sl]

                    z1 = psz.tile([H1, B], F32, tag="z")
                    nc.tensor.matmul(z1[:], w1[0:2, :], yTs, start=True, stop=True)
                    u1 = psu.tile([H1, B], F32, tag="u")
                    nc.tensor.matmul(u1[:], w1[0:2, :], vTs, start=True, stop=True)
                    h1 = hp.tile([H1, B], F32R, tag="h1")
                    t1 = hp.tile([H1, B], F32R, tag="t1")
                    layer(z1[:], u1[:], b1c, b1h, h1[:], t1[:], H1)

                    z2 = psz.tile([H2, B], F32, tag="z")
                    nc.tensor.matmul(z2[:], w2[:], h1[:], start=True, stop=True)
                    u2 = psu.tile([H2, B], F32, tag="u")
                    nc.tensor.matmul(u2[:], w2[:], t1[:], start=True, stop=True)
                    h2 = hp.tile([H2, B], F32R, tag="h2")
                    t2 = hp.tile([H2, B], F32R, tag="t2")
                    layer(z2[:], u2[:], b2sb, b2h, h2[:], t2[:], H2)

                    z3 = ps3.tile([H3, B], F32, tag="z3")
                    nc.tensor.matmul(z3[:], w3[:], h2[:], start=True, stop=True)
                    u3 = psu3.tile([H3, B], F32, tag="u3")
                    nc.tensor.matmul(u3[:], w3[:], t2[:], start=True, stop=True)
                    h3 = hp.tile([H3, B], F32R, tag="h3")
                    t3 = hp.tile([H3, B], F32R, tag="t3")
                    layer(z3[:], u3[:], b3sb, b3h, h3[:], t3[:], H3)

                    # L4
                    dyp = pst.tile([D, B], F32, tag="dyp")
                    nc.tensor.matmul(dyp[:], w4[:], h3[:], start=True, stop=True)
                    jvp = psd.tile([D, B], F32, tag="jvp")
                    nc.tensor.matmul(jvp[:], w4[:], t3[:], start=True, stop=True)

                    # dy (+b4) into the chunk staging tile
                    nc.scalar.activation(
                        dyT[:, sl], dyp[:], AF.Identity, bias=b4sb[:], scale=1.0
                    )
                    # -div = (-1,-1) . (v * Jv); the div matmul reuses jvp's bank
                    pv = hp.tile([D, B], F32R, tag="pv")
                    nc.vector.tensor_tensor(pv[:], jvp[:], vTs, AluOpType.mult)
                    nc.tensor.matmul(jvp[0:1, :], negones[:], pv[:], start=True, stop=True)
                    nc.scalar.activation(ndT[:, sl], jvp[0:1, :], AF.Copy)

                nc.sync.dma_start(
                    dyv[r0 : r0 + CB, :].rearrange("n d -> d n"), dyT[:]
                )
                nc.sync.dma_start(
                    ndv[r0 : r0 + CB, :].rearrange("n d -> d n"), ndT[:]
                )

    nc.compile()
    return nc


def kernel(**inputs):
    y = np.ascontiguousarray(np.asarray(inputs["y"], dtype=np.float32))
    v = np.ascontiguousarray(np.asarray(inputs["v"], dtype=np.float32))
    t = np.asarray(inputs["t"], dtype=np.float32).reshape(1, 1)
    n = y.shape[0]
    rows = n // N_CORES

    key = rows
    if key not in _CACHE:
        _CACHE[key] = build_module(rows)
    nc = _CACHE[key]

    W1 = np.ascontiguousarray(inputs["W1"], dtype=np.float32)
    b1c = (np.asarray(inputs["b1"], np.float32) + float(t.reshape(())) * W1[2, :]).astype(np.float32)
    b2c = np.ascontiguousarray(inputs["b2"], dtype=np.float32)
    b3c = np.ascontiguousarray(inputs["b3"], dtype=np.float32)
    shared = {
        "W1": W1,
        "W2": np.ascontiguousarray(inputs["W2"], dtype=np.float32),
        "W3": np.ascontiguousarray(inputs["W3"], dtype=np.float32),
        "W4": np.ascontiguousarray(inputs["W4"], dtype=np.float32),
        "b1c": b1c, "b1h": (0.5 * b1c).astype(np.float32),
        "b2c": b2c, "b2h": (0.5 * b2c).astype(np.float32),
        "b3c": b3c, "b3h": (0.5 * b3c).astype(np.float32),
        "b4": np.ascontiguousarray(inputs["b4"], dtype=np.float32),
    }
    in_maps = []
    for c in range(N_CORES):
        sl = slice(c * rows, (c + 1) * rows)
        in_maps.append(
            {"y": np.ascontiguousarray(y[sl]), "v": np.ascontiguousarray(v[sl]), **shared}
        )

    res = run_bass_kernel_spmd(nc, in_maps, core_ids=list(range(N_CORES)))
    dy = np.concatenate([res.results[c]["dy"] for c in range(N_CORES)], axis=0)
    nd = np.concatenate([res.results[c]["nd"] for c in range(N_CORES)], axis=0)
    return dy, nd


def _make_in_maps(inputs):
    y = np.ascontiguousarray(np.asarray(inputs["y"], dtype=np.float32))
    v = np.ascontiguousarray(np.asarray(inputs["v"], dtype=np.float32))
    t = np.asarray(inputs["t"], dtype=np.float32).reshape(1, 1)
    rows = y.shape[0] // N_CORES
    W1 = np.ascontiguousarray(inputs["W1"], dtype=np.float32)
    b1c = (np.asarray(inputs["b1"], np.float32) + float(t.reshape(())) * W1[2, :]).astype(np.float32)
    b2c = np.ascontiguousarray(inputs["b2"], dtype=np.float32)
    b3c = np.ascontiguousarray(inputs["b3"], dtype=np.float32)
    shared = {
        "W1": W1,
        "W2": np.ascontiguousarray(inputs["W2"], dtype=np.float32),
        "W3": np.ascontiguousarray(inputs["W3"], dtype=np.float32),
        "W4": np.ascontiguousarray(inputs["W4"], dtype=np.float32),
        "b1c": b1c, "b1h": (0.5 * b1c).astype(np.float32),
        "b2c": b2c, "b2h": (0.5 * b2c).astype(np.float32),
        "b3c": b3c, "b3h": (0.5 * b3c).astype(np.float32),
        "b4": np.ascontiguousarray(inputs["b4"], dtype=np.float32),
    }
    in_maps = []
    for c in range(N_CORES):
        sl = slice(c * rows, (c + 1) * rows)
        in_maps.append(
            {"y": np.ascontiguousarray(y[sl]), "v": np.ascontiguousarray(v[sl]), **shared}
        )
    return in_maps


# revision 13
# speedup vs baseline: 4.0502x; 4.0502x over previous
"""Trainium2 Bass kernel for nn_CNF_ODE_46093589021148.

Computes dy = mlp(cat[y,t]) and -div = -sum(v * (J^T v), -1) for a 4-layer
SiLU MLP (3->128->128->64->2), N=524288 rows, data-parallel over 8 cores.

Math: since div = v^T (J^T v) = v^T (J v), forward-mode JVP with tangent
u0 = [v, 0] replaces the VJP: per layer
    z = h_prev @ W + b ; h = silu(z) ; u = (u_prev @ W) * silu'(z)
then dy = h3 @ W4 + b4, Jv = u3 @ W4, -div = -(v0*Jv0 + v1*Jv1).

silu'(z) = 0.5*(1+th) + h*0.5*(1-th) with th = tanh(z/2), h = silu(z):
both served by the ACT engine's silu_and_others table set (one table load).

Layout: activations are feature-major [feat, batch] tiles of B=512 columns.
The scalar t is folded into layer-1 bias: b1c = b1 + t*W1[2,:], so L1 runs
with K=2 (y rows only); tangent L1 likewise (tangent of the t-column is 0).
L3 primal+tangent share one PSUM bank (partitions 0-63 / 64-127); L4
primal/tangent outputs share one bank at partitions 0-1 / 64-65.
"""

import numpy as np

import concourse.bacc as bacc
import concourse.mybir as mybir
from concourse import tile
from concourse.alu_op_type import AluOpType
from concourse.bass_utils import run_bass_kernel_spmd

F32 = mybir.dt.float32
F32R = mybir.dt.float32r
AF = mybir.ActivationFunctionType

N_CORES = 8
N_TOTAL = 524288
R = N_TOTAL // N_CORES  # rows per core
B = 512                 # batch tile (PSUM bank = 512 fp32)
G = 4                   # tiles per DMA chunk
H1, H2, H3, D = 128, 128, 64, 2

_CACHE = {}


def _r(ap):
    return ap.bitcast(F32R)


def build_module(rows=R):
    n_tiles = rows // B
    n_chunks = max(1, n_tiles // G)
    g = n_tiles // n_chunks

    nc = bacc.Bacc("TRN2", target_bir_lowering=False)

    y = nc.dram_tensor("yT", [2, rows], F32R, kind="ExternalInput")
    v = nc.dram_tensor("vT", [2, rows], F32R, kind="ExternalInput")
    W1 = nc.dram_tensor("W1", [3, H1], F32, kind="ExternalInput")
    W2 = nc.dram_tensor("W2", [H1, H2], F32, kind="ExternalInput")
    W3 = nc.dram_tensor("W3", [H2, H3], F32, kind="ExternalInput")
    W4 = nc.dram_tensor("W4", [H3, D], F32, kind="ExternalInput")
    b1c_d = nc.dram_tensor("b1c", [H1], F32, kind="ExternalInput")
    b1h_d = nc.dram_tensor("b1h", [H1], F32, kind="ExternalInput")
    b2c_d = nc.dram_tensor("b2c", [H2], F32, kind="ExternalInput")
    b2h_d = nc.dram_tensor("b2h", [H2], F32, kind="ExternalInput")
    b3c_d = nc.dram_tensor("b3c", [H3], F32, kind="ExternalInput")
    b3h_d = nc.dram_tensor("b3h", [H3], F32, kind="ExternalInput")
    b4_d = nc.dram_tensor("b4", [D], F32, kind="ExternalInput")
    dy = nc.dram_tensor("dyT", [2, rows], F32, kind="ExternalOutput")
    nd = nc.dram_tensor("ndT", [1, rows], F32, kind="ExternalOutput")

    with tile.TileContext(nc) as tc:
        with (
            tc.tile_pool(name="wp", bufs=1) as wp,
            tc.tile_pool(name="iop", bufs=2) as iop,
            tc.tile_pool(name="outp", bufs=2) as outp,
            tc.tile_pool(name="hp", bufs=2) as hp,
            tc.tile_pool(name="accp", bufs=2) as accp,
            tc.tile_pool(name="psz", bufs=2, space="PSUM") as psz,
            tc.tile_pool(name="psu", bufs=2, space="PSUM") as psu,
            tc.tile_pool(name="ps3", bufs=1, space="PSUM") as ps3,
            tc.tile_pool(name="psu3", bufs=1, space="PSUM") as psu3,
            tc.tile_pool(name="pst", bufs=1, space="PSUM") as pst,
            tc.tile_pool(name="psd", bufs=1, space="PSUM") as psd,
        ):
            # ---- one-time setup -------------------------------------------
            def wload(dt, shape):
                wr = wp.tile(shape, F32R, tag=dt.name + "_r")
                nc.sync.dma_start(wr[:], dt.ap().bitcast(F32R))
                return wr

            w1 = wload(W1, [3, H1])
            w2 = wload(W2, [H1, H2])
            w3 = wload(W3, [H2, H3])
            w4 = wload(W4, [H3, D])

            def bias_tile(dt, P):
                tl = wp.tile([P, 1], F32, tag=dt.name + "_sb")
                nc.sync.dma_start(tl[:], dt.ap().unsqueeze(-1))
                return tl

            b1c = bias_tile(b1c_d, H1)
            b1h = bias_tile(b1h_d, H1)
            b2sb = bias_tile(b2c_d, H2)
            b2h = bias_tile(b2h_d, H2)
            b3sb = bias_tile(b3c_d, H3)
            b3h = bias_tile(b3h_d, H3)
            b4sb = bias_tile(b4_d, D)

            negones_f = wp.tile([D, 1], F32)
            nc.gpsimd.memset(negones_f[:], -1.0)
            negones = wp.tile([D, 1], F32R)
            nc.vector.tensor_copy(negones[:], negones_f[:])

            yv = y.ap()
            vv = v.ap()
            dyv = dy.ap()
            ndv = nd.ap()

            def layer(z_ps, u_ps, bc, bh, out_h, out_t, P):
                """z,u: PSUM [P,B] views -> h=silu(z+b) into out_h (SBUF),
                tangent u*silu'(z+b) into out_t (SBUF)."""
                th = hp.tile([P, B], F32, tag=f"th{P}")
                nc.scalar.activation(out_h, z_ps, AF.Silu, bias=bc[:], scale=1.0)
                nc.scalar.activation(th[:], z_ps, AF.Tanh, bias=bh[:], scale=0.5)
                m = hp.tile([P, B], F32, tag=f"m{P}")
                acc = accp.tile([P, 1], F32, tag=f"acc{P}")
                nc.vector.affine_mul_reduce(
                    m[:], acc[:], th[:], out_h, scale=-0.5, bias=0.5
                )
                d = hp.tile([P, B], F32, tag=f"d{P}")
                nc.vector.affine_then_add(d[:], th[:], m[:], scale=0.5, bias=0.5)
                nc.vector.tensor_tensor(out_t, u_ps, d[:], AluOpType.mult)

            # ---- main loop ------------------------------------------------
            for c in range(n_chunks):
                CB = g * B
                r0 = c * CB
                yT = iop.tile([2, CB], F32R, tag="yT")
                nc.sync.dma_start(yT[:], yv[:, r0 : r0 + CB])
                vT = iop.tile([2, CB], F32R, tag="vT")
                nc.sync.dma_start(vT[:], vv[:, r0 : r0 + CB])
                dyT = outp.tile([2, CB], F32, tag="dyT")
                ndT = outp.tile([1, CB], F32, tag="ndT")

                for j in range(g):
                    sl = slice(j * B, (j + 1) * B)
                    yTs = yT[:, sl]
                    vTs = vT[:, sl]

                    z1 = psz.tile([H1, B], F32, tag="z")
                    nc.tensor.matmul(z1[:], w1[0:2, :], yTs, start=True, stop=True)
                    u1 = psu.tile([H1, B], F32, tag="u")
                    nc.tensor.matmul(u1[:], w1[0:2, :], vTs, start=True, stop=True)
                    h1 = hp.tile([H1, B], F32R, tag="h1")
                    t1 = hp.tile([H1, B], F32R, tag="t1")
                    layer(z1[:], u1[:], b1c, b1h, h1[:], t1[:], H1)

                    z2 = psz.tile([H2, B], F32, tag="z")
                    nc.tensor.matmul(z2[:], w2[:], h1[:], start=True, stop=True)
                    u2 = psu.tile([H2, B], F32, tag="u")
                    nc.tensor.matmul(u2[:], w2[:], t1[:], start=True, stop=True)
                    h2 = hp.tile([H2, B], F32R, tag="h2")
                    t2 = hp.tile([H2, B], F32R, tag="t2")
                    layer(z2[:], u2[:], b2sb, b2h, h2[:], t2[:], H2)

                    z3 = ps3.tile([H3, B], F32, tag="z3")
                    nc.tensor.matmul(z3[:], w3[:], h2[:], start=True, stop=True)
                    u3 = psu3.tile([H3, B], F32, tag="u3")
                    nc.tensor.matmul(u3[:], w3[:], t2[:], start=True, stop=True)
                    h3 = hp.tile([H3, B], F32R, tag="h3")
                    t3 = hp.tile([H3, B], F32R, tag="t3")
                    layer(z3[:], u3[:], b3sb, b3h, h3[:], t3[:], H3)

                    # L4
                    dyp = pst.tile([D, B], F32, tag="dyp")
                    nc.tensor.matmul(dyp[:], w4[:], h3[:], start=True, stop=True)
                    jvp = psd.tile([D, B], F32, tag="jvp")
                    nc.tensor.matmul(jvp[:], w4[:], t3[:], start=True, stop=True)

                    # dy (+b4) into the chunk staging tile
                    nc.scalar.activation(
                        dyT[:, sl], dyp[:], AF.Identity, bias=b4sb[:], scale=1.0
                    )
                    # -div = (-1,-1) . (v * Jv); the div matmul reuses jvp's bank
                    pv = hp.tile([D, B], F32R, tag="pv")
                    nc.vector.tensor_tensor(pv[:], jvp[:], vTs, AluOpType.mult)
                    nc.tensor.matmul(jvp[0:1, :], negones[:], pv[:], start=True, stop=True)
                    nc.scalar.activation(ndT[:, sl], jvp[0:1, :], AF.Copy)

                nc.sync.dma_start(dyv[:, r0 : r0 + CB], dyT[:])
                nc.sync.dma_start(ndv[:, r0 : r0 + CB], ndT[:])

    nc.compile()
    return nc


def kernel(**inputs):
    y = np.ascontiguousarray(np.asarray(inputs["y"], dtype=np.float32))
    v = np.ascontiguousarray(np.asarray(inputs["v"], dtype=np.float32))
    t = np.asarray(inputs["t"], dtype=np.float32).reshape(1, 1)
    n = y.shape[0]
    rows = n // N_CORES

    key = rows
    if key not in _CACHE:
        _CACHE[key] = build_module(rows)
    nc = _CACHE[key]

    W1 = np.ascontiguousarray(inputs["W1"], dtype=np.float32)
    b1c = (np.asarray(inputs["b1"], np.float32) + float(t.reshape(())) * W1[2, :]).astype(np.float32)
    b2c = np.ascontiguousarray(inputs["b2"], dtype=np.float32)
    b3c = np.ascontiguousarray(inputs["b3"], dtype=np.float32)
    shared = {
        "W1": W1,
        "W2": np.ascontiguousarray(inputs["W2"], dtype=np.float32),
        "W3": np.ascontiguousarray(inputs["W3"], dtype=np.float32),
        "W4": np.ascontiguousarray(inputs["W4"], dtype=np.float32),
        "b1c": b1c, "b1h": (0.5 * b1c).astype(np.float32),
        "b2c": b2c, "b2h": (0.5 * b2c).astype(np.float32),
        "b3c": b3c, "b3h": (0.5 * b3c).astype(np.float32),
        "b4": np.ascontiguousarray(inputs["b4"], dtype=np.float32),
    }
    yT = np.ascontiguousarray(y.T)
    vT = np.ascontiguousarray(v.T)
    in_maps = []
    for c in range(N_CORES):
        sl = slice(c * rows, (c + 1) * rows)
        in_maps.append(
            {
                "yT": np.ascontiguousarray(yT[:, sl]),
                "vT": np.ascontiguousarray(vT[:, sl]),
                **shared,
            }
        )

    res = run_bass_kernel_spmd(nc, in_maps, core_ids=list(range(N_CORES)))
    dy = np.concatenate(
        [np.ascontiguousarray(res.results[c]["dyT"].T) for c in range(N_CORES)], axis=0
    )
    nd = np.concatenate(
        [res.results[c]["ndT"].reshape(-1, 1) for c in range(N_CORES)], axis=0
    )
    return dy, nd


def _make_in_maps(inputs):
    y = np.ascontiguousarray(np.asarray(inputs["y"], dtype=np.float32))
    v = np.ascontiguousarray(np.asarray(inputs["v"], dtype=np.float32))
    t = np.asarray(inputs["t"], dtype=np.float32).reshape(1, 1)
    rows = y.shape[0] // N_CORES
    W1 = np.ascontiguousarray(inputs["W1"], dtype=np.float32)
    b1c = (np.asarray(inputs["b1"], np.float32) + float(t.reshape(())) * W1[2, :]).astype(np.float32)
    b2c = np.ascontiguousarray(inputs["b2"], dtype=np.float32)
    b3c = np.ascontiguousarray(inputs["b3"], dtype=np.float32)
    shared = {
        "W1": W1,
        "W2": np.ascontiguousarray(inputs["W2"], dtype=np.float32),
        "W3": np.ascontiguousarray(inputs["W3"], dtype=np.float32),
        "W4": np.ascontiguousarray(inputs["W4"], dtype=np.float32),
        "b1c": b1c, "b1h": (0.5 * b1c).astype(np.float32),
        "b2c": b2c, "b2h": (0.5 * b2c).astype(np.float32),
        "b3c": b3c, "b3h": (0.5 * b3c).astype(np.float32),
        "b4": np.ascontiguousarray(inputs["b4"], dtype=np.float32),
    }
    yT = np.ascontiguousarray(y.T)
    vT = np.ascontiguousarray(v.T)
    in_maps = []
    for c in range(N_CORES):
        sl = slice(c * rows, (c + 1) * rows)
        in_maps.append(
            {
                "yT": np.ascontiguousarray(yT[:, sl]),
                "vT": np.ascontiguousarray(vT[:, sl]),
                **shared,
            }
        )
    return in_maps
